# revision 26
# baseline (speedup 1.0000x reference)
"""Expert-parallel MoE MLP kernel for TRN2 (8 NeuronCores, 1 expert/core).

Math per core (expert e):
    h   = gelu(x_e @ w1_e + b1_e)      x_e: [4096, 1024], w1_e: [1024, 4096]
    out = h @ w2_e + b2_e              w2_e: [4096, 1024]

Host-side prep (inside kernel(), part of the sharding step): x_e is
transposed to [D, tok] and cast to bf16, w1/w2 are cast to bf16. On
device the kernel is then pure matmul work:
  - mm1: stationary = w1 tile [128(D), 128(H)], moving = xT tile
    [128(D), 512(tok)] -> PSUM hT tile; ACT applies exact-erf GELU (+b1
    as per-partition bias) PSUM->SBUF bf16.
  - mm2: stationary = hT tile [128(H), 128(tok)], moving = w2 tile
    [128(H), 512(D)] -> PSUM out tile (fp32); DVE adds b2, DMA out.
Weights live in SBUF as 32+32 independent [128,1024] bf16 tiles loaded
over both DMA paths (HWDGE via sync + SWDGE via gpsimd) in an order that
lets chunk-0 mm1 start after ~2MB of DMA instead of the full preload.
All matmuls accumulate fp32 in PSUM; bf16 only rounds the inputs.
"""

import numpy as np
import ml_dtypes

import concourse.bacc as bacc
import concourse.bass as bass
import concourse.mybir as mybir
import concourse.tile as tile
from concourse import bass_utils

P = 128
D = 1024
H = 4096
NTOK = 4096  # B*N per expert
NCORES = 8
CHUNK = 512  # tokens per pipeline chunk
NCHUNK = NTOK // CHUNK
F32 = mybir.dt.float32
BF16 = mybir.dt.bfloat16
GELU = mybir.ActivationFunctionType.Gelu

DK = D // P    # 8   k-tiles of D
HM = H // P    # 32  tiles of H
HQ = 4         # W1 loaded in 4 column quarters (1024 wide)
TSUB = CHUNK // P  # 4 token subtiles per chunk
DC = D // 512  # 2   512-wide output column chunks


def build_program(act=GELU):
    nc = bacc.Bacc("TRN2", target_bir_lowering=False, debug=False,
                   num_devices=NCORES)

    xt_d = nc.dram_tensor("xt", (D, NTOK), BF16, kind="ExternalInput").ap()
    w1 = nc.dram_tensor("w1", (D, H), BF16, kind="ExternalInput").ap()
    # biases arrive pre-arranged from the host: b1 as [128, 32] (H on
    # partitions), b2 replicated to [128, 1024] — plain contiguous DMAs
    b1 = nc.dram_tensor("b1", (P, HM), F32, kind="ExternalInput").ap()
    w2 = nc.dram_tensor("w2", (H, D), BF16, kind="ExternalInput").ap()
    b2 = nc.dram_tensor("b2", (P, D), F32, kind="ExternalInput").ap()
    out = nc.dram_tensor("out", (NTOK, D), F32, kind="ExternalOutput").ap()

    with tile.TileContext(nc) as tc:
        with (
            tc.tile_pool(name="consts", bufs=1) as consts,
            tc.tile_pool(name="weights", bufs=1) as wpool,
            tc.tile_pool(name="xt", bufs=2) as xtp,
            tc.tile_pool(name="ht", bufs=1) as htp,
            tc.tile_pool(name="outp", bufs=4) as outp,
            tc.tile_pool(name="p1", bufs=3, space="PSUM") as p1p,
            tc.tile_pool(name="p2", bufs=4, space="PSUM") as p2p,
        ):
            def load_xt_chunk(c):
                xt = [xtp.tile([P, CHUNK], BF16, tag=f"xt{dk}",
                               name=f"xt{dk}_c{c}") for dk in range(DK)]
                for dk in range(DK):
                    eng = nc.sync if dk % 2 == 0 else nc.gpsimd
                    eng.dma_start(
                        xt[dk], xt_d[dk * P:(dk + 1) * P,
                                     c * CHUNK:(c + 1) * CHUNK])
                return xt

            # b1 first — tiny, and the first gelu (which drains the p1
            # PSUM pool) needs it
            b1_sb = consts.tile([P, HM], F32, tag="b1")
            nc.sync.dma_start(b1_sb, b1)


            # chunk-0 activations interleaved dk-major with the first W1
            # column-quarter so mm1's k-loop unblocks progressively
            w1t = [[None] * HQ for _ in range(DK)]

            def alloc_w1(dk, hq):
                t = wpool.tile([P, D], BF16, tag=f"w1_{dk}_{hq}",
                               name=f"w1_{dk}_{hq}")
                w1t[dk][hq] = t
                return t

            xt0 = [xtp.tile([P, CHUNK], BF16, tag=f"xt{dk}",
                            name=f"xt{dk}_c0") for dk in range(DK)]
            for dk in range(DK):
                e0 = nc.sync if dk % 2 == 0 else nc.gpsimd
                e1 = nc.gpsimd if dk % 2 == 0 else nc.sync
                e0.dma_start(xt0[dk], xt_d[dk * P:(dk + 1) * P, 0:CHUNK])
                e1.dma_start(alloc_w1(dk, 0), w1[dk * P:(dk + 1) * P, 0:D])

            # remaining W1 quarters, alternating DMA queues
            for hq in range(1, HQ):
                for dk in range(DK):
                    eng = nc.sync if (hq * DK + dk) % 2 == 0 else nc.gpsimd
                    eng.dma_start(alloc_w1(dk, hq),
                                  w1[dk * P:(dk + 1) * P,
                                     hq * D:(hq + 1) * D])

            # b2 (replicated on host) — needed from the first out tile ~95us
            b2_rep = consts.tile([P, D], F32, tag="b2rep")
            nc.gpsimd.dma_start(b2_rep, b2)

            w2t = []
            for hk in range(HM):
                t = wpool.tile([P, D], BF16, tag=f"w2_{hk}",
                               name=f"w2_{hk}")
                w2t.append(t)
                eng = nc.sync if hk % 2 == 0 else nc.gpsimd
                eng.dma_start(t, w2[hk * P:(hk + 1) * P, :])

            # ---- main pipeline over token chunks ----
            for c in range(NCHUNK):
                # xT tiles straight from DRAM (bf16), double-buffered so
                # chunk c+1 prefetches during chunk c.
                xt = xt0 if c == 0 else load_xt_chunk(c)

                # mm1 + gelu -> hT tiles (bf16, H on partitions)
                ht = [htp.tile([P, CHUNK], BF16, tag=f"ht{hm}",
                               name=f"ht{hm}_c{c}") for hm in range(HM)]
                for hm in range(HM):
                    p1 = p1p.tile([P, CHUNK], F32, tag="p1",
                                  name=f"p1_c{c}h{hm}")
                    hcol = (hm % (HM // HQ)) * P
                    for dk in range(DK):
                        nc.tensor.matmul(
                            p1,
                            w1t[dk][hm // (HM // HQ)][:, hcol:hcol + P],
                            xt[dk],
                            start=(dk == 0), stop=(dk == DK - 1))
                    nc.scalar.activation(ht[hm], p1, act,
                                         bias=b1_sb[:, hm:hm + 1], scale=1.0)

                # mm2 (+b2) -> out
                for ts in range(TSUB):
                    p2s = [p2p.tile([P, 512], F32, tag="p2",
                                    name=f"p2_c{c}t{ts}d{dc}")
                           for dc in range(DC)]
                    for hk in range(HM):
                        lhsT = ht[hk][:, ts * P:(ts + 1) * P]
                        for dc in range(DC):
                            nc.tensor.matmul(
                                p2s[dc], lhsT,
                                w2t[hk][:, dc * 512:(dc + 1) * 512],
                                start=(hk == 0), stop=(hk == HM - 1))
                    r0 = c * CHUNK + ts * P
                    for dc in range(DC):
                        ot = outp.tile([P, 512], F32, tag="ot",
                                       name=f"ot_c{c}t{ts}d{dc}")
                        nc.vector.tensor_add(
                            ot, p2s[dc], b2_rep[:, dc * 512:(dc + 1) * 512])
                        oeng = nc.sync if (ts + dc) % 2 == 0 else nc.gpsimd
                        oeng.dma_start(
                            out[r0:r0 + P, dc * 512:(dc + 1) * 512], ot)

    nc.compile()
    return nc


_CACHE: dict = {}


def _program():
    if "nc" not in _CACHE:
        _CACHE["nc"] = build_program()
    return _CACHE["nc"]


def _in_maps(x, w1, b1, w2, b2):
    x = np.asarray(x, dtype=np.float32)
    w1 = np.asarray(w1, dtype=np.float32)
    b1 = np.asarray(b1, dtype=np.float32)
    w2 = np.asarray(w2, dtype=np.float32)
    b2 = np.asarray(b2, dtype=np.float32)
    bf = ml_dtypes.bfloat16
    maps = []
    for e in range(NCORES):
        xt_e = np.ascontiguousarray(
            x[:, e].reshape(NTOK, D).T.astype(bf))  # [D, NTOK] bf16
        maps.append({
            "xt": xt_e,
            "w1": np.ascontiguousarray(w1[e].astype(bf)),
            "b1": np.ascontiguousarray(b1[e].reshape(HM, P).T),
            "w2": np.ascontiguousarray(w2[e].astype(bf)),
            "b2": np.ascontiguousarray(
                np.broadcast_to(b2[e], (P, D))),
        })
    return maps


def _install_ntff_hook_shim():
    """Provide antenv.axon_hooks if the image lacks it, wiring the NTFF
    profile hook straight to libaxon_pjrt.so (mirrors trn_agent_boot)."""
    import sys
    try:
        from antenv.axon_hooks import get_axon_ntff_profile_hook  # noqa: F401
        return
    except ImportError:
        pass
    import contextlib
    import ctypes
    import types

    import antenv

    hook = None
    so_path = "/opt/axon/libaxon_pjrt.so"
    try:
        lib = ctypes.CDLL(so_path)
        if hasattr(lib, "axon_start_nrt_profile"):
            lib.axon_start_nrt_profile.argtypes = [
                ctypes.POINTER(ctypes.c_int64), ctypes.c_size_t]
            lib.axon_start_nrt_profile.restype = ctypes.c_int64
            lib.axon_stop_nrt_profile.argtypes = [ctypes.c_char_p]
            lib.axon_stop_nrt_profile.restype = ctypes.c_int64

            @contextlib.contextmanager
            def _hook(output_dir, device_ids):
                import jax
                jax.devices()
                if device_ids:
                    ids = (ctypes.c_int64 * len(device_ids))(*device_ids)
                    rc = lib.axon_start_nrt_profile(ids, len(device_ids))
                else:
                    rc = lib.axon_start_nrt_profile(None, 0)
                if rc != 0:
                    raise RuntimeError(f"axon_start_nrt_profile rc={rc}")
                try:
                    yield
                finally:
                    n = lib.axon_stop_nrt_profile(str(output_dir).encode())
                    print(f"ntff profile: {n} file(s) -> {output_dir}")

            hook = _hook
    except OSError:
        pass

    mod = types.ModuleType("antenv.axon_hooks")
    mod._hook = hook
    mod.get_axon_ntff_profile_hook = lambda: mod._hook
    mod.set_axon_ntff_profile_hook = lambda h: setattr(mod, "_hook", h)
    sys.modules["antenv.axon_hooks"] = mod
    antenv.axon_hooks = mod


def run_spmd(x, w1, b1, w2, b2, trace=False):
    if trace:
        _install_ntff_hook_shim()
    nc = _program()
    res = bass_utils.run_bass_kernel_spmd(
        nc, _in_maps(x, w1, b1, w2, b2), core_ids=list(range(NCORES)),
        trace=trace)
    outs = [r["out"].reshape(4, 1024, D) for r in res.results]
    full = np.stack(outs, axis=1).astype(np.float32)  # [4, 8, 1024, 1024]
    return full, res


def kernel(x, w1, b1, w2, b2):
    full, _ = run_spmd(x, w1, b1, w2, b2)
    return full



# revision 27
# speedup vs baseline: 1.0054x; 1.0054x over previous
"""Expert-parallel MoE MLP kernel for TRN2 (8 NeuronCores, 1 expert/core).

Math per core (expert e):
    h   = gelu(x_e @ w1_e + b1_e)      x_e: [4096, 1024], w1_e: [1024, 4096]
    out = h @ w2_e + b2_e              w2_e: [4096, 1024]

Host-side prep (inside kernel(), part of the sharding step): x_e is
transposed to [D, tok] and cast to bf16, w1/w2 are cast to bf16. On
device the kernel is then pure matmul work:
  - mm1: stationary = w1 tile [128(D), 128(H)], moving = xT tile
    [128(D), 512(tok)] -> PSUM hT tile; ACT applies exact-erf GELU (+b1
    as per-partition bias) PSUM->SBUF bf16.
  - mm2: stationary = hT tile [128(H), 128(tok)], moving = w2 tile
    [128(H), 512(D)] -> PSUM out tile (fp32); DVE adds b2, DMA out.
Weights live in SBUF as 32+32 independent [128,1024] bf16 tiles loaded
over both DMA paths (HWDGE via sync + SWDGE via gpsimd) in an order that
lets chunk-0 mm1 start after ~2MB of DMA instead of the full preload.
All matmuls accumulate fp32 in PSUM; bf16 only rounds the inputs.
"""

import numpy as np
import ml_dtypes

import concourse.bacc as bacc
import concourse.bass as bass
import concourse.mybir as mybir
import concourse.tile as tile
from concourse import bass_utils

P = 128
D = 1024
H = 4096
NTOK = 4096  # B*N per expert
NCORES = 8
CHUNK = 512  # tokens per pipeline chunk
NCHUNK = NTOK // CHUNK
F32 = mybir.dt.float32
BF16 = mybir.dt.bfloat16
GELU = mybir.ActivationFunctionType.Gelu

DK = D // P    # 8   k-tiles of D
HM = H // P    # 32  tiles of H
HQ = 4         # W1 loaded in 4 column quarters (1024 wide)
TSUB = CHUNK // P  # 4 token subtiles per chunk
DC = D // 512  # 2   512-wide output column chunks


def build_program(act=GELU):
    nc = bacc.Bacc("TRN2", target_bir_lowering=False, debug=False,
                   num_devices=NCORES)

    xt_d = nc.dram_tensor("xt", (D, NTOK), BF16, kind="ExternalInput").ap()
    w1 = nc.dram_tensor("w1", (D, H), BF16, kind="ExternalInput").ap()
    # biases arrive pre-arranged from the host: b1 as [128, 32] (H on
    # partitions), b2 replicated to [128, 1024] — plain contiguous DMAs
    b1 = nc.dram_tensor("b1", (P, HM), F32, kind="ExternalInput").ap()
    w2 = nc.dram_tensor("w2", (H, D), BF16, kind="ExternalInput").ap()
    b2 = nc.dram_tensor("b2", (P, D), F32, kind="ExternalInput").ap()
    out = nc.dram_tensor("out", (NTOK, D), F32, kind="ExternalOutput").ap()

    with tile.TileContext(nc) as tc:
        with (
            tc.tile_pool(name="consts", bufs=1) as consts,
            tc.tile_pool(name="weights", bufs=1) as wpool,
            tc.tile_pool(name="xt", bufs=2) as xtp,
            tc.tile_pool(name="ht", bufs=1) as htp,
            tc.tile_pool(name="outp", bufs=4) as outp,
            tc.tile_pool(name="p1", bufs=4, space="PSUM") as p1p,
            tc.tile_pool(name="p2", bufs=4, space="PSUM") as p2p,
        ):
            def load_xt_chunk(c):
                xt = [xtp.tile([P, CHUNK], BF16, tag=f"xt{dk}",
                               name=f"xt{dk}_c{c}") for dk in range(DK)]
                for dk in range(DK):
                    eng = nc.sync if dk % 2 == 0 else nc.gpsimd
                    eng.dma_start(
                        xt[dk], xt_d[dk * P:(dk + 1) * P,
                                     c * CHUNK:(c + 1) * CHUNK])
                return xt

            # b1 first — tiny, and the first gelu (which drains the p1
            # PSUM pool) needs it
            b1_sb = consts.tile([P, HM], F32, tag="b1")
            nc.sync.dma_start(b1_sb, b1)


            # chunk-0 activations interleaved dk-major with the first W1
            # column-quarter so mm1's k-loop unblocks progressively
            w1t = [[None] * HQ for _ in range(DK)]

            def alloc_w1(dk, hq):
                t = wpool.tile([P, D], BF16, tag=f"w1_{dk}_{hq}",
                               name=f"w1_{dk}_{hq}")
                w1t[dk][hq] = t
                return t

            xt0 = [xtp.tile([P, CHUNK], BF16, tag=f"xt{dk}",
                            name=f"xt{dk}_c0") for dk in range(DK)]
            for dk in range(DK):
                e0 = nc.sync if dk % 2 == 0 else nc.gpsimd
                e1 = nc.gpsimd if dk % 2 == 0 else nc.sync
                e0.dma_start(xt0[dk], xt_d[dk * P:(dk + 1) * P, 0:CHUNK])
                e1.dma_start(alloc_w1(dk, 0), w1[dk * P:(dk + 1) * P, 0:D])

            # remaining W1 quarters, alternating DMA queues
            for hq in range(1, HQ):
                for dk in range(DK):
                    eng = nc.sync if (hq * DK + dk) % 2 == 0 else nc.gpsimd
                    eng.dma_start(alloc_w1(dk, hq),
                                  w1[dk * P:(dk + 1) * P,
                                     hq * D:(hq + 1) * D])

            # b2 (replicated on host) — needed from the first out tile ~95us
            b2_rep = consts.tile([P, D], F32, tag="b2rep")
            nc.gpsimd.dma_start(b2_rep, b2)

            w2t = []
            for hk in range(HM):
                t = wpool.tile([P, D], BF16, tag=f"w2_{hk}",
                               name=f"w2_{hk}")
                w2t.append(t)
                eng = nc.sync if hk % 2 == 0 else nc.gpsimd
                eng.dma_start(t, w2[hk * P:(hk + 1) * P, :])

            # ---- main pipeline over token chunks ----
            for c in range(NCHUNK):
                # xT tiles straight from DRAM (bf16), double-buffered so
                # chunk c+1 prefetches during chunk c.
                xt = xt0 if c == 0 else load_xt_chunk(c)

                # mm1 + gelu -> hT tiles (bf16, H on partitions)
                ht = [htp.tile([P, CHUNK], BF16, tag=f"ht{hm}",
                               name=f"ht{hm}_c{c}") for hm in range(HM)]
                for hm in range(HM):
                    p1 = p1p.tile([P, CHUNK], F32, tag="p1",
                                  name=f"p1_c{c}h{hm}")
                    hcol = (hm % (HM // HQ)) * P
                    for dk in range(DK):
                        nc.tensor.matmul(
                            p1,
                            w1t[dk][hm // (HM // HQ)][:, hcol:hcol + P],
                            xt[dk],
                            start=(dk == 0), stop=(dk == DK - 1))
                    nc.scalar.activation(ht[hm], p1, act,
                                         bias=b1_sb[:, hm:hm + 1], scale=1.0)

                # mm2 (+b2) -> out
                for ts in range(TSUB):
                    p2s = [p2p.tile([P, 512], F32, tag="p2",
                                    name=f"p2_c{c}t{ts}d{dc}")
                           for dc in range(DC)]
                    for hk in range(HM):
                        lhsT = ht[hk][:, ts * P:(ts + 1) * P]
                        for dc in range(DC):
                            nc.tensor.matmul(
                                p2s[dc], lhsT,
                                w2t[hk][:, dc * 512:(dc + 1) * 512],
                                start=(hk == 0), stop=(hk == HM - 1))
                    r0 = c * CHUNK + ts * P
                    for dc in range(DC):
                        ot = outp.tile([P, 512], F32, tag="ot",
                                       name=f"ot_c{c}t{ts}d{dc}")
                        nc.vector.tensor_add(
                            ot, p2s[dc], b2_rep[:, dc * 512:(dc + 1) * 512])
                        oeng = nc.sync if (ts + dc) % 2 == 0 else nc.gpsimd
                        oeng.dma_start(
                            out[r0:r0 + P, dc * 512:(dc + 1) * 512], ot)

    nc.compile()
    return nc


_CACHE: dict = {}


def _program():
    if "nc" not in _CACHE:
        _CACHE["nc"] = build_program()
    return _CACHE["nc"]


def _in_maps(x, w1, b1, w2, b2):
    x = np.asarray(x, dtype=np.float32)
    w1 = np.asarray(w1, dtype=np.float32)
    b1 = np.asarray(b1, dtype=np.float32)
    w2 = np.asarray(w2, dtype=np.float32)
    b2 = np.asarray(b2, dtype=np.float32)
    bf = ml_dtypes.bfloat16
    maps = []
    for e in range(NCORES):
        xt_e = np.ascontiguousarray(
            x[:, e].reshape(NTOK, D).T.astype(bf))  # [D, NTOK] bf16
        maps.append({
            "xt": xt_e,
            "w1": np.ascontiguousarray(w1[e].astype(bf)),
            "b1": np.ascontiguousarray(b1[e].reshape(HM, P).T),
            "w2": np.ascontiguousarray(w2[e].astype(bf)),
            "b2": np.ascontiguousarray(
                np.broadcast_to(b2[e], (P, D))),
        })
    return maps


def _install_ntff_hook_shim():
    """Provide antenv.axon_hooks if the image lacks it, wiring the NTFF
    profile hook straight to libaxon_pjrt.so (mirrors trn_agent_boot)."""
    import sys
    try:
        from antenv.axon_hooks import get_axon_ntff_profile_hook  # noqa: F401
        return
    except ImportError:
        pass
    import contextlib
    import ctypes
    import types

    import antenv

    hook = None
    so_path = "/opt/axon/libaxon_pjrt.so"
    try:
        lib = ctypes.CDLL(so_path)
        if hasattr(lib, "axon_start_nrt_profile"):
            lib.axon_start_nrt_profile.argtypes = [
                ctypes.POINTER(ctypes.c_int64), ctypes.c_size_t]
            lib.axon_start_nrt_profile.restype = ctypes.c_int64
            lib.axon_stop_nrt_profile.argtypes = [ctypes.c_char_p]
            lib.axon_stop_nrt_profile.restype = ctypes.c_int64

            @contextlib.contextmanager
            def _hook(output_dir, device_ids):
                import jax
                jax.devices()
                if device_ids:
                    ids = (ctypes.c_int64 * len(device_ids))(*device_ids)
                    rc = lib.axon_start_nrt_profile(ids, len(device_ids))
                else:
                    rc = lib.axon_start_nrt_profile(None, 0)
                if rc != 0:
                    raise RuntimeError(f"axon_start_nrt_profile rc={rc}")
                try:
                    yield
                finally:
                    n = lib.axon_stop_nrt_profile(str(output_dir).encode())
                    print(f"ntff profile: {n} file(s) -> {output_dir}")

            hook = _hook
    except OSError:
        pass

    mod = types.ModuleType("antenv.axon_hooks")
    mod._hook = hook
    mod.get_axon_ntff_profile_hook = lambda: mod._hook
    mod.set_axon_ntff_profile_hook = lambda h: setattr(mod, "_hook", h)
    sys.modules["antenv.axon_hooks"] = mod
    antenv.axon_hooks = mod


def run_spmd(x, w1, b1, w2, b2, trace=False):
    if trace:
        _install_ntff_hook_shim()
    nc = _program()
    res = bass_utils.run_bass_kernel_spmd(
        nc, _in_maps(x, w1, b1, w2, b2), core_ids=list(range(NCORES)),
        trace=trace)
    outs = [r["out"].reshape(4, 1024, D) for r in res.results]
    full = np.stack(outs, axis=1).astype(np.float32)  # [4, 8, 1024, 1024]
    return full, res


def kernel(x, w1, b1, w2, b2):
    full, _ = run_spmd(x, w1, b1, w2, b2)
    return full

